# revision 5
# baseline (speedup 1.0000x reference)
"""MoE kernel for Trainium2 (8 NeuronCores, expert-parallel), bf16 compute.

Problem: nn_MoE_78151224918194
  hidden_states [4, 2048, 2048] f32 -> out [4, 2048, 2048] f32
  E=8 routed experts (top-2, softmax-renormalized), I=1408,
  plus a shared SwiGLU FFN with IS=2816.

Strategy (PE-bound at 78.6 TF/s; bf16 runs the 128x128 PE at the same
1 cycle/row as fp32r while halving DMA and SBUF):
  - Gate (softmax + top-2 + renormalize) on host with jax-on-CPU, exactly
    mirroring the reference, so expert selection matches bitwise.
  - Expert-parallel: core c runs expert c's FFN over the tokens routed to
    it (host-gathered, padded to C = max count rounded up to 2 -- exact
    padding beats the old 256-round, cutting ~4.5% of wasted columns).
  - Shared FFN is token-parallel: core c also runs the shared expert
    (split into two I=1408 halves, jobs A and B) on token slice
    [c*1024, (c+1)*1024).
  - All tensors bf16 (x, weights, activations, outputs); matmul
    accumulation in fp32 PSUM. End-to-end rel err ~4e-3.
  - Column blocks sized so every 512-wide PSUM subtile stays >= 256 and
    each block's full-weight-set stream stays covered by compute;
    smallest block first to shorten pipeline fill.
  - gate/up psum pools triple-buffered (pg+pu+po = 8 PSUM banks); down
    weights DMA'd 4 m-tiles per transfer.
  - Host combine: y = concat(shared slices) + scatter-add of weighted
    routed outputs.
"""

import numpy as np
import ml_dtypes

import concourse.bacc as bacc
import concourse.mybir as mybir
import concourse.tile as tile
from concourse.bass_utils import run_bass_kernel_spmd

P = 128
H = 2048
I = 1408
E = 8
TOP_K = 2
IS = 2816
KH = 16       # k-tiles over H
KI = 11       # i-tiles over I
SHARED_SLICE = 1024
NSUB = 512
TB_MAX = 2048

F32 = mybir.dt.float32
BF16 = mybir.dt.bfloat16
NPBF = ml_dtypes.bfloat16

PG_BUFS = 3   # gate/up psum double->triple buffering (3+3+2 = 8 banks)
PO_BUFS = 2
W_BUFS = 2

LAST_RESULTS = None
_BUILD_CACHE = {}


def _blocks(n, min_block=256, tb_max=TB_MAX):
    """Partition n columns into blocks, smallest first (shortens the
    pipeline fill of the first block). A block's 512-wide subtile split
    must not produce a subtile below 256, so block % 512 must be 0 or
    >= 256; each block must also be >= min_block so its weight-set DMA
    stream stays covered by compute. Prefers fewer, larger blocks."""
    ok = lambda b: min_block <= b <= tb_max and (b % 512 == 0 or b % 512 >= 256)
    assert n >= min_block, n
    memo = {}

    def solve(rem):
        if rem == 0:
            return []
        if rem in memo:
            return memo[rem]
        best = None
        for b in range(min(tb_max, rem), min_block - 1, -32):
            if not ok(b):
                continue
            sub = solve(rem - b)
            if sub is not None and (best is None or len(sub) + 1 < len(best)):
                best = [b] + sub
                if len(best) == (rem + tb_max - 1) // tb_max:
                    break
        memo[rem] = best
        return best

    out = solve(n)
    assert out is not None, (n, min_block)
    return sorted(out)


def _emit_ffn(nc, sbuf, psum, x_ap, wgu_ap, wd_ap, out_ap, n_tok, jtag):
    """One bf16 SwiGLU FFN job: out = ((silu(x@wg) * (x@wu)) @ wd).T

    x_ap:   [P, KH, n]        bf16 (k-tile-major token columns)
    wgu_ap: [KI, P, 2, KH, P] bf16 (gate+up, stationary tiles)
    wd_ap:  [4, P, 4, KI, P]  bf16 (down, 4 m-tiles per DMA chunk)
    out_ap: [H, n]            bf16
    """
    silu = mybir.ActivationFunctionType.Silu
    b0 = 0
    for TB in _blocks(n_tok):
        nsubs = [(s, min(NSUB, TB - s)) for s in range(0, TB, NSUB)]
        bt = f"{jtag}b{b0}"
        # i=0 weight DMAs are emitted BEFORE the x tiles: the SP sequencer
        # dispatches DMAs in emission order (~0.7us each), so putting 18
        # dispatches ahead of the first gate weight tile would stall the
        # first matmul group ~11us at kernel start.
        wgu0 = []
        for half in range(2):
            w0 = sbuf.tile([P, KH, P], BF16, name=f"w{half}{bt}i0",
                           tag=("wg" if half == 0 else "wu"), bufs=W_BUFS)
            nc.sync.dma_start(w0[:], wgu_ap[0, :, half])
            wgu0.append(w0)
        x_tiles = []
        for k in range(KH):
            xk = sbuf.tile([P, TB], BF16, name=f"x{bt}k{k}", tag=f"x{k}",
                           bufs=1)
            nc.sync.dma_start(xk[:], x_ap[:, k, b0:b0 + TB])
            x_tiles.append(xk)
        act_tiles = []
        for i in range(KI):
            if i == 0:
                wg_sb, wu_sb = wgu0
            else:
                wg_sb = sbuf.tile([P, KH, P], BF16, name=f"wg{bt}i{i}",
                                  tag="wg", bufs=W_BUFS)
                wu_sb = sbuf.tile([P, KH, P], BF16, name=f"wu{bt}i{i}",
                                  tag="wu", bufs=W_BUFS)
                nc.sync.dma_start(wg_sb[:], wgu_ap[i, :, 0])
                nc.sync.dma_start(wu_sb[:], wgu_ap[i, :, 1])
            act = sbuf.tile([P, TB], BF16, name=f"act{bt}i{i}", tag=f"act{i}",
                            bufs=1)
            for s, w in nsubs:
                pg = psum.tile([P, NSUB], F32, name=f"pg{bt}i{i}s{s}", tag="pg",
                               bufs=PG_BUFS)
                pu = psum.tile([P, NSUB], F32, name=f"pu{bt}i{i}s{s}", tag="pu",
                               bufs=PG_BUFS)
                for k in range(KH):
                    nc.tensor.matmul(pg[:, :w], wg_sb[:, k],
                                     x_tiles[k][:, s:s + w],
                                     start=(k == 0), stop=(k == KH - 1))
                for k in range(KH):
                    nc.tensor.matmul(pu[:, :w], wu_sb[:, k],
                                     x_tiles[k][:, s:s + w],
                                     start=(k == 0), stop=(k == KH - 1))
                tmp = sbuf.tile([P, NSUB], F32, name=f"tmp{bt}i{i}s{s}",
                                tag="silu", bufs=3)
                nc.scalar.activation(tmp[:, :w], pg[:, :w], silu)
                nc.vector.tensor_tensor(act[:, s:s + w], tmp[:, :w], pu[:, :w],
                                        mybir.AluOpType.mult)
            act_tiles.append(act)
        for mc in range(4):
            wd_sb = sbuf.tile([P, 4, KI, P], BF16, name=f"wd{bt}c{mc}",
                              tag="wd", bufs=W_BUFS)
            nc.sync.dma_start(wd_sb[:], wd_ap[mc])
            for mm in range(4):
                m = mc * 4 + mm
                for s, w in nsubs:
                    po = psum.tile([P, NSUB], F32, name=f"po{bt}m{m}s{s}",
                                   tag="po", bufs=PO_BUFS)
                    for i in range(KI):
                        nc.tensor.matmul(po[:, :w], wd_sb[:, mm, i],
                                         act_tiles[i][:, s:s + w],
                                         start=(i == 0), stop=(i == KI - 1))
                    ot = sbuf.tile([P, NSUB], BF16, name=f"ot{bt}m{m}s{s}",
                                   tag="ot", bufs=3)
                    nc.vector.tensor_copy(ot[:, :w], po[:, :w])
                    nc.sync.dma_start(
                        out_ap[m * P:(m + 1) * P, b0 + s:b0 + s + w], ot[:, :w])
        b0 += TB


def _build(C, reps=1, loop=0):
    nc = bacc.Bacc(trn_type="TRN2", target_bir_lowering=False, debug=False)
    GU_SHAPE = [KI, P, 2, KH, P]
    DC_SHAPE = [4, P, 4, KI, P]
    xr = nc.dram_tensor("xr", [P, KH, C], BF16, kind="ExternalInput")
    xs = nc.dram_tensor("xs", [P, KH, SHARED_SLICE], BF16,
                        kind="ExternalInput")
    w_in = {}
    for nm in ("rgu", "agu", "bgu"):
        w_in[nm] = nc.dram_tensor(nm, GU_SHAPE, BF16, kind="ExternalInput")
    for nm in ("rd", "ad", "bd"):
        w_in[nm] = nc.dram_tensor(nm, DC_SHAPE, BF16, kind="ExternalInput")
    yr = nc.dram_tensor("yr", [H, C], BF16, kind="ExternalOutput")
    ya = nc.dram_tensor("ya", [H, SHARED_SLICE], BF16, kind="ExternalOutput")
    yb = nc.dram_tensor("yb", [H, SHARED_SLICE], BF16, kind="ExternalOutput")

    with tile.TileContext(nc) as tc:
        with (
            tc.tile_pool(name="sbuf", bufs=2) as sbuf,
            tc.tile_pool(name="psum", bufs=2, space="PSUM") as psum,
        ):
            def body():
                for r in range(reps):
                    _emit_ffn(nc, sbuf, psum, xr.ap(), w_in["rgu"].ap(),
                              w_in["rd"].ap(), yr.ap(), C, f"r{r}_")
                    _emit_ffn(nc, sbuf, psum, xs.ap(), w_in["agu"].ap(),
                              w_in["ad"].ap(), ya.ap(), SHARED_SLICE, f"a{r}_")
                    _emit_ffn(nc, sbuf, psum, xs.ap(), w_in["bgu"].ap(),
                              w_in["bd"].ap(), yb.ap(), SHARED_SLICE, f"b{r}_")
            if loop:
                with tc.For_i(0, loop, 1):
                    body()
            else:
                body()
    nc.compile()
    return nc


def _get_nc(C, reps=1, loop=0):
    key = (C, reps, loop)
    if key not in _BUILD_CACHE:
        _BUILD_CACHE[key] = _build(C, reps, loop)
    return _BUILD_CACHE[key]


def _gate_host(x, gate_w):
    """Softmax + top-2 + renormalize, mirroring the jax reference on CPU."""
    try:
        import jax
        cpu = jax.devices("cpu")[0]
        with jax.default_device(cpu):
            import jax.numpy as jnp
            logits = jnp.asarray(x) @ jnp.asarray(gate_w).T
            scores = jax.nn.softmax(logits, axis=-1)
            topk_w, topk_idx = jax.lax.top_k(scores, TOP_K)
            topk_w = topk_w / (jnp.sum(topk_w, axis=-1, keepdims=True) + 1e-20)
        return np.asarray(topk_w), np.asarray(topk_idx)
    except Exception:
        logits = x @ gate_w.T
        m = logits.max(axis=-1, keepdims=True)
        ex = np.exp(logits - m)
        scores = ex / ex.sum(axis=-1, keepdims=True)
        order = np.argsort(-scores, axis=-1, kind="stable")
        topk_idx = order[:, :TOP_K]
        topk_w = np.take_along_axis(scores, topk_idx, axis=-1)
        topk_w = topk_w / (topk_w.sum(axis=-1, keepdims=True) + 1e-20)
        return topk_w.astype(np.float32), topk_idx.astype(np.int32)


def _x_pack(xt):
    """[H, n] f32 -> [P, KH, n] bf16 (k-tile-major)."""
    n = xt.shape[1]
    return np.ascontiguousarray(
        xt.reshape(KH, P, n).transpose(1, 0, 2)).astype(NPBF)


def _wlayout_ud(w):
    # [H, I_like] -> [KI', P(H), KH, P(I)] stationary tiles for gate/up
    ki = w.shape[1] // P
    return np.ascontiguousarray(w.reshape(KH, P, ki, P).transpose(2, 1, 0, 3))


def _wlayout_down(w):
    # [I_like, H] -> [KH, P(I), KI', P(H)]
    ki = w.shape[0] // P
    return np.ascontiguousarray(w.reshape(ki, P, KH, P).transpose(2, 1, 0, 3))


def _w_pack(wg, wu, wd, prefix, out):
    """gate+up merged [KI,P,2,KH,P]; down chunked [4,P,4,KI,P]; bf16."""
    gl = _wlayout_ud(wg)
    ul = _wlayout_ud(wu)
    out[prefix + "gu"] = np.ascontiguousarray(
        np.stack([gl, ul], axis=2)).astype(NPBF)
    dl = _wlayout_down(wd)
    out[prefix + "d"] = np.ascontiguousarray(
        dl.reshape(4, 4, P, KI, P).transpose(0, 2, 1, 3, 4)).astype(NPBF)


def _prepare(hidden_states, gate_w, we_gate, we_up, we_down,
             ws_gate, ws_up, ws_down):
    B, S, h = hidden_states.shape
    x = np.ascontiguousarray(hidden_states.reshape(-1, h)).astype(np.float32)
    topk_w, topk_idx = _gate_host(x, gate_w)

    idx_lists, w_lists = [], []
    for e in range(E):
        mask = (topk_idx == e)
        idx = np.nonzero(mask.any(axis=1))[0]
        we = np.where(mask, topk_w, 0.0).sum(axis=1)[idx].astype(np.float32)
        idx_lists.append(idx)
        w_lists.append(we)
    max_cnt = max(len(ix) for ix in idx_lists)
    C = max(256, ((max_cnt + 1) // 2) * 2)

    shared = {}
    _w_pack(ws_gate[:, :I], ws_up[:, :I], ws_down[:I], "a", shared)
    _w_pack(ws_gate[:, I:], ws_up[:, I:], ws_down[I:], "b", shared)

    in_maps = []
    for c in range(E):
        idx = idx_lists[c]
        m = dict(shared)
        xr = np.zeros((H, C), dtype=np.float32)
        xr[:, :len(idx)] = x[idx].T
        m["xr"] = _x_pack(xr)
        m["xs"] = _x_pack(np.ascontiguousarray(
            x[c * SHARED_SLICE:(c + 1) * SHARED_SLICE].T))
        _w_pack(we_gate[c], we_up[c], we_down[c], "r", m)
        in_maps.append(m)
    return in_maps, idx_lists, w_lists, C


def _combine(results, idx_lists, w_lists, T):
    y = np.empty((T, H), dtype=np.float32)
    for c in range(E):
        sh = (results[c]["ya"].astype(np.float32)
              + results[c]["yb"].astype(np.float32))
        y[c * SHARED_SLICE:(c + 1) * SHARED_SLICE] = sh.T
    for c in range(E):
        idx = idx_lists[c]
        y[idx] += w_lists[c][:, None] * \
            results[c]["yr"][:, :len(idx)].T.astype(np.float32)
    return y


def kernel(hidden_states, gate_w, we_gate, we_up, we_down,
           ws_gate, ws_up, ws_down):
    global LAST_RESULTS
    B, S, h = hidden_states.shape
    in_maps, idx_lists, w_lists, C = _prepare(
        hidden_states, gate_w, we_gate, we_up, we_down,
        ws_gate, ws_up, ws_down)
    nc = _get_nc(C)
    res = run_bass_kernel_spmd(nc, in_maps, core_ids=list(range(E)))
    LAST_RESULTS = res
    y = _combine(res.results, idx_lists, w_lists, B * S)
    return y.reshape(B, S, h).astype(hidden_states.dtype)
